# revision 13
# baseline (speedup 1.0000x reference)
"""Trainium2 Bass kernel for ContractiveInvertibleGNN feed-forward.

Math (reference, with group_mask == I_32):
  out[b,i] = f_i( sum_j W_adj[j,i] * g_j(X[b,j]) )
where g_j: R -> R^32 and f_i: R^32 -> R are slices of two shared MLPs
(64->128->128->32 with a residual middle block, LeakyReLU 0.01):
  g: H1 = lrelu(X[b,j]*U_j + C1_j); H2 = H1 + lrelu(H1@W2g + b2g)
     X_emb = H2 @ W3g + b3g
  f: Hf1 = lrelu(X_aggr@Wf1x + C2_i); Hf2 = Hf1 + lrelu(Hf1@Wf2 + bf2)
     out_i = Hf2 . V_i (+ bf3_i)
with per-node constants U_j = g_W1[j,:], C1_j = emb_j@g_W1[32:]+g_b1,
C2_i = emb_i@f_W1[32:]+f_b1 (+ (sum_j W_adj[j,i])*g_b3@f_W1[:32]),
V_i = f_W3[:,i].

Sharding: pure data-parallel over batch across 8 cores (2048 rows each).

Engine balance (the point of this implementation): activations are bf16 in
SBUF (psum stays f32). LeakyReLU work is split by column ranges across the
Activation engine (fused scale/bias lrelu), DVE (mult by alpha) and Pool
(max) so no single engine serializes the kernel. The f-phase residual
H2f = Hf1 + lrelu(...) is materialized with one DVE add so the final V-dot
needs 4 accumulating matmuls instead of 8. Matmuls run on bf16 operands
(1 cycle/row, same as f32r) which also halves DMA and removes the
f32->f32r staging copies of the earlier version.

On-chip layout (per core): node-major columns. g-phase runs per node j over
[128, 2048] tiles; X_emb assembled as Xe[(c,d), (j,t)] with c = batch
quarter stacked on partition groups; DVE transpose -> Xt[(c,j),(t,d)];
block-diag(W_adj) matmul aggregates over j; DVE transpose back ->
Xa[(c,d),(i,t)]; f-phase per node i with padded stationaries selecting
partition group c; final dot with V_i via a [128,32] stationary that also
routes batch quarter c to psum row c.
"""

import os
import sys

import numpy as np

for _p in ("/opt/trn_rl_repo", "/root/.axon_site/_ro/trn_rl_repo"):
    if os.path.isdir(_p) and _p not in sys.path:
        sys.path.insert(0, _p)

N = 32          # nodes
D = 32          # processed dim (== N, group_mask = I)
A = 128         # hidden width
B = 16384       # batch
NCORES = 8
BC = B // NCORES        # 2048 rows per core
CH = 512                # matmul free-dim chunk
NCH = BC // CH          # 4 chunks (partition-group stacking factor)
ALPHA = 0.01

# Column splits: how many of each per-node activation's 2048 columns run on
# the Activation engine. The rest: h1's tail runs as DVE-scale + Pool
# mult/max (SBUF-sourced, Pool-legal); t2/tf tails run as DVE mult+max
# pairs (PSUM-sourced; GPSIMD cannot access PSUM). The f-phase residual
# Hf2 = hf1 + tf is materialized (DVE/Pool split) so V-dot needs only 4
# accumulating matmuls.
H1A = 1152              # h1 lrelu Act cols; rest DVE-z + Pool mult/max
T2A = 1536              # t2 lrelu Act cols; rest DVE mult+max pair
HF1A = 2048             # hf1 lrelu: all on Act (needs per-node bias C2_i)
TFA = 1344              # tf lrelu Act cols; rest DVE pair
HF2D = 768              # hf2-add cols on DVE; rest on Pool


def _build_program(zero_b2=True):
    from contextlib import ExitStack

    from concourse import bacc, mybir, tile

    f32 = mybir.dt.float32
    bf16 = mybir.dt.bfloat16
    LRELU = mybir.ActivationFunctionType.Lrelu
    ALU_MULT = mybir.AluOpType.mult
    ALU_ADD = mybir.AluOpType.add
    ALU_MAX = mybir.AluOpType.max
    ALU_MIN = mybir.AluOpType.min

    nc = bacc.Bacc("TRN2", target_bir_lowering=False, debug=False)

    def din(name, shape, dt):
        return nc.dram_tensor(
            name, list(shape), dt, kind="ExternalInput"
        ).ap()

    xt_d = din("XT", (N, BC), bf16)
    gw2_d = din("GW2", (A, A), bf16)
    fw2_d = din("FW2", (A, A), bf16)
    gw3p_d = din("GW3P", (A, NCH * A), bf16)   # col-block c: g_W3 at cols 32c..
    fw1p_d = din("FW1P", (A, NCH * A), bf16)   # row-block c: f_W1[:32] rows 32c..
    bd_d = din("BD", (A, A), bf16)             # kron(I4, W_adj)
    u_d = din("U", (A, N), f32)
    c1_d = din("C1", (A, N), f32)
    c2_d = din("C2", (A, N), f32)
    gb2_d = din("GB2", (A, 1), f32)
    fb2_d = din("FB2", (A, 1), f32)
    vp_d = din("VP", (A, (N + 1) * D), bf16)   # [:, (i+1)*D] = V_i[a]; else 0
    out_d = nc.dram_tensor("OUT", [N, BC], f32, kind="ExternalOutput").ap()

    HCH = 2 * CH        # 1024: half of a node's batch columns
    T2AH = T2A // 2     # Act cols of t2 per half
    TFAH = TFA // 2     # Act cols of tf per half

    with tile.TileContext(nc) as tc, ExitStack() as ctx:
        const = ctx.enter_context(tc.tile_pool(name="const", bufs=1))
        xep = ctx.enter_context(tc.tile_pool(name="xep", bufs=1))
        workp = ctx.enter_context(tc.tile_pool(name="work", bufs=3))
        scrp = ctx.enter_context(tc.tile_pool(name="scr", bufs=3))
        outp = ctx.enter_context(tc.tile_pool(name="outs", bufs=2))
        ppA = ctx.enter_context(tc.tile_pool(name="ppA", bufs=2, space="PSUM"))
        ppB = ctx.enter_context(tc.tile_pool(name="ppB", bufs=2, space="PSUM"))
        ppR = ctx.enter_context(tc.tile_pool(name="ppR", bufs=2, space="PSUM"))

        def load_const(ap_dram, shape):
            t = const.tile(list(shape), ap_dram.dtype,
                           tag=f"c_{ap_dram.tensor.name}")
            nc.sync.dma_start(t[:, :], ap_dram)
            return t

        gw2_s = load_const(gw2_d, (A, A))
        fw2_s = load_const(fw2_d, (A, A))
        gw3p_s = load_const(gw3p_d, (A, NCH * A))
        fw1p_s = load_const(fw1p_d, (A, NCH * A))
        bd_s = load_const(bd_d, (A, A))
        u_s = load_const(u_d, (A, N))
        c1_s = load_const(c1_d, (A, N))
        c2_s = load_const(c2_d, (A, N))
        gb2_s = load_const(gb2_d, (A, 1))
        fb2_s = load_const(fb2_d, (A, 1))
        vp_s = load_const(vp_d, (A, (N + 1) * D))

        # Xe[(c,d), (j,t)] = X_emb[d, j, c*CH+t]
        xe = xep.tile([A, N * CH], bf16, tag="xe")

        # ---------------- g phase: one node j per iteration ----------------
        for j in range(N):
            xbc = workp.tile([A, BC], bf16, tag="xbc")
            nc.sync.dma_start(
                xbc[:, :], xt_d[j : j + 1, :].partition_broadcast(A)
            )
            h1 = workp.tile([A, BC], bf16, tag="h1")
            nc.scalar.activation(
                h1[:, :H1A], xbc[:, :H1A], LRELU,
                bias=c1_s[:, j : j + 1], scale=u_s[:, j : j + 1], alpha=ALPHA,
            )
            if H1A < BC:
                # DVE computes z = x*u + c1 (bf16, SBUF); Pool (SBUF-only,
                # no float max in its ISA) finishes the lrelu as
                # z + (alpha-1)*min(z, 0).
                dcols = BC - H1A
                zt = scrp.tile([A, dcols], bf16, tag="zt")
                mt = scrp.tile([A, dcols], bf16, tag="mt")
                nc.vector.tensor_scalar(zt[:, :], xbc[:, H1A:],
                                        u_s[:, j : j + 1], c1_s[:, j : j + 1],
                                        ALU_MULT, ALU_ADD)
                nc.gpsimd.tensor_scalar(mt[:, :], zt[:, :], 0.0, ALPHA - 1.0,
                                        ALU_MIN, ALU_MULT)
                nc.gpsimd.tensor_tensor(h1[:, H1A:], zt[:, :], mt[:, :],
                                        ALU_ADD)
            t2 = workp.tile([A, BC], bf16, tag="t2")
            for h in range(2):  # halves of 1024 cols
                pa = ppA.tile([A, HCH], f32, tag="pA")
                for q in range(2):
                    sl = slice(h * HCH + q * CH, h * HCH + (q + 1) * CH)
                    nc.tensor.matmul(
                        pa[:, q * CH : (q + 1) * CH], gw2_s[:, :],
                        h1[:, sl], start=True, stop=True,
                    )
                off = h * HCH
                if T2AH:
                    nc.scalar.activation(
                        t2[:, off : off + T2AH], pa[:, :T2AH], LRELU,
                        bias=gb2_s[:, 0:1], alpha=ALPHA,
                    )
                pcols = HCH - T2AH
                if pcols:
                    # PSUM-sourced tail: DVE-only pair (GPSIMD can't read
                    # PSUM).
                    m2 = scrp.tile([A, pcols], bf16, tag="m2")
                    if zero_b2:
                        nc.vector.tensor_scalar(
                            m2[:, :], pa[:, T2AH:], ALPHA, None, ALU_MULT)
                        nc.vector.tensor_tensor(
                            t2[:, off + T2AH : off + HCH], pa[:, T2AH:],
                            m2[:, :], ALU_MAX)
                    else:
                        z2 = scrp.tile([A, pcols], bf16, tag="z2")
                        nc.vector.tensor_scalar(
                            z2[:, :], pa[:, T2AH:], gb2_s[:, 0:1], None,
                            ALU_ADD)
                        nc.gpsimd.tensor_scalar(
                            m2[:, :], z2[:, :], 0.0, ALPHA - 1.0,
                            ALU_MIN, ALU_MULT)
                        nc.gpsimd.tensor_tensor(
                            t2[:, off + T2AH : off + HCH], z2[:, :],
                            m2[:, :], ALU_ADD)
            # X_emb = g_W3^T @ (H1 + lrelu(.)) via 8 accumulating matmuls,
            # chunk c routed to psum rows 32c by the padded stationary.
            pm3 = ppB.tile([A, CH], f32, tag="pB")
            for c in range(NCH):
                lt = gw3p_s[:, c * A : (c + 1) * A]
                sl = slice(c * CH, (c + 1) * CH)
                nc.tensor.matmul(pm3[:, :], lt, h1[:, sl],
                                 start=(c == 0), stop=False)
                nc.tensor.matmul(pm3[:, :], lt, t2[:, sl],
                                 start=False, stop=(c == NCH - 1))
            nc.vector.tensor_copy(xe[:, j * CH : (j + 1) * CH], pm3[:, :])

        # ---------------- aggregation ----------------
        # T1: Xe[(c,d),(j,t)] -> Xt[(c,j),(t,d)]
        xt3 = xe.rearrange("p (j t) -> p j t", j=N).transpose([0, 2, 1])
        xtile = xep.tile([A, CH * D], bf16, tag="xt")
        xto = xtile.rearrange("p (t d) -> p t d", d=D)
        TS = 8  # split into 8 ops for overlap
        tstep = CH // TS
        for s in range(TS):
            nc.vector.transpose(
                xto[:, s * tstep : (s + 1) * tstep, :],
                xt3[:, s * tstep : (s + 1) * tstep, :],
            )
        # agg windows + T2-back: psum[(c,i),(t16,d)] -> Xa[(c,d),(i,t)].
        # StreamTranspose can't convert dtypes, so the idle Act engine
        # copies psum f32 -> bf16 first (Copy shares the Lrelu table).
        COPY = mybir.ActivationFunctionType.Copy
        xa = xep.tile([A, N * CH], bf16, tag="xa")
        xa3 = xa.rearrange("p (i t) -> p i t", i=N).transpose([0, 2, 1])
        WT = CH // D  # 16 t per window
        for w in range(CH // WT):  # 32 windows
            pg = ppB.tile([A, CH], f32, tag="pB")
            nc.tensor.matmul(
                pg[:, :], bd_s[:, :],
                xtile[:, w * CH : (w + 1) * CH], start=True, stop=True,
            )
            pgb = scrp.tile([A, CH], bf16, tag="pgb")
            nc.scalar.activation(pgb[:, :], pg[:, :], COPY)
            nc.vector.transpose(
                xa3[:, w * WT : (w + 1) * WT, :],
                pgb.rearrange("p (t d) -> p t d", d=D)[:, :, :],
            )

        # ---------------- f phase: one node i per iteration ----------------
        for i in range(N):
            rhs = xa[:, i * CH : (i + 1) * CH]
            hf1 = workp.tile([A, BC], bf16, tag="hf1")
            for h in range(2):
                pa = ppA.tile([A, HCH], f32, tag="pA")
                for q in range(2):
                    c = h * 2 + q
                    nc.tensor.matmul(
                        pa[:, q * CH : (q + 1) * CH],
                        fw1p_s[:, c * A : (c + 1) * A], rhs,
                        start=True, stop=True,
                    )
                nc.scalar.activation(
                    hf1[:, h * HCH : (h + 1) * HCH], pa[:, :], LRELU,
                    bias=c2_s[:, i : i + 1], alpha=ALPHA,
                )
            tf = workp.tile([A, BC], bf16, tag="tf")
            for h in range(2):
                pa = ppA.tile([A, HCH], f32, tag="pA")
                for q in range(2):
                    c = h * 2 + q
                    nc.tensor.matmul(
                        pa[:, q * CH : (q + 1) * CH], fw2_s[:, :],
                        hf1[:, c * CH : (c + 1) * CH], start=True, stop=True,
                    )
                off = h * HCH
                if TFAH:
                    nc.scalar.activation(
                        tf[:, off : off + TFAH], pa[:, :TFAH], LRELU,
                        bias=fb2_s[:, 0:1], alpha=ALPHA,
                    )
                pcols = HCH - TFAH
                if pcols:
                    # PSUM-sourced tail: DVE-only pair (GPSIMD can't read
                    # PSUM).
                    mf = scrp.tile([A, pcols], bf16, tag="mf")
                    if zero_b2:
                        nc.vector.tensor_scalar(
                            mf[:, :], pa[:, TFAH:], ALPHA, None, ALU_MULT)
                        nc.vector.tensor_tensor(
                            tf[:, off + TFAH : off + HCH], pa[:, TFAH:],
                            mf[:, :], ALU_MAX)
                    else:
                        zf = scrp.tile([A, pcols], bf16, tag="zf")
                        nc.vector.tensor_scalar(
                            zf[:, :], pa[:, TFAH:], fb2_s[:, 0:1], None,
                            ALU_ADD)
                        nc.gpsimd.tensor_scalar(
                            mf[:, :], zf[:, :], 0.0, ALPHA - 1.0,
                            ALU_MIN, ALU_MULT)
                        nc.gpsimd.tensor_tensor(
                            tf[:, off + TFAH : off + HCH], zf[:, :],
                            mf[:, :], ALU_ADD)
            # Residual add in SBUF (legal for GPSIMD): split DVE/Pool.
            hf2 = workp.tile([A, BC], bf16, tag="hf2")
            nc.vector.tensor_tensor(hf2[:, :HF2D], hf1[:, :HF2D],
                                    tf[:, :HF2D], ALU_ADD)
            nc.gpsimd.tensor_tensor(hf2[:, HF2D:], hf1[:, HF2D:],
                                    tf[:, HF2D:], ALU_ADD)
            pr = ppR.tile([D, CH], f32, tag="pR")
            for c in range(NCH):
                base = (i + 1) * D - c
                lt = vp_s[:, base : base + D]
                nc.tensor.matmul(pr[:, :], lt, hf2[:, c * CH : (c + 1) * CH],
                                 start=(c == 0), stop=(c == NCH - 1))
            osb = outp.tile([NCH, CH], f32, tag="o")
            nc.vector.tensor_copy(osb[:, :], pr[:NCH, :])
            nc.sync.dma_start(
                out_d[i : i + 1, :].rearrange("o (c t) -> (o c) t", c=NCH),
                osb[:, :],
            )

    nc.compile()
    return nc


_NC_CACHE = {}


def _get_program(zero_b2=True):
    if zero_b2 not in _NC_CACHE:
        _NC_CACHE[zero_b2] = _build_program(zero_b2)
    return _NC_CACHE[zero_b2]


def _bf16(x):
    import ml_dtypes
    return np.asarray(x, np.float32).astype(ml_dtypes.bfloat16)


def _host_consts(W, embeddings, g_W1, g_b1, g_W2, g_b2, g_W3, g_b3,
                 f_W1, f_b1, f_W2, f_b2, f_W3, f_b3):
    f = np.float32
    W_adj = (W * (1.0 - np.eye(N, dtype=f))).astype(f)
    U = np.ascontiguousarray(g_W1[:D].T, dtype=f)                    # [A, N]
    C1 = np.ascontiguousarray((embeddings @ g_W1[D:] + g_b1).T, f)   # [A, N]
    s = W_adj.sum(axis=0)                                            # [N]
    C2 = (embeddings @ f_W1[D:] + f_b1 + np.outer(s, g_b3 @ f_W1[:D]))
    C2 = np.ascontiguousarray(C2.T, dtype=f)                         # [A, N]
    GW3P = np.zeros((A, NCH * A), f)
    FW1P = np.zeros((A, NCH * A), f)
    for c in range(NCH):
        GW3P[:, c * A + c * D : c * A + (c + 1) * D] = g_W3
        FW1P[c * D : (c + 1) * D, c * A : (c + 1) * A] = f_W1[:D]
    BD = np.kron(np.eye(NCH, dtype=f), W_adj).astype(f)
    VP = np.zeros((A, (N + 1) * D), f)
    for i in range(N):
        VP[:, (i + 1) * D] = f_W3[:, i]
    return {
        "GW2": _bf16(g_W2),
        "FW2": _bf16(f_W2),
        "GW3P": _bf16(GW3P), "FW1P": _bf16(FW1P), "BD": _bf16(BD),
        "U": U, "C1": C1, "C2": C2,
        "GB2": np.ascontiguousarray(g_b2.reshape(A, 1), f),
        "FB2": np.ascontiguousarray(f_b2.reshape(A, 1), f),
        "VP": _bf16(VP),
    }


def _kernel_numpy(X, W, embeddings, g_W1, g_b1, g_W2, g_b2, g_W3, g_b3,
                  f_W1, f_b1, f_W2, f_b2, f_W3, f_b3, group_mask):
    # general fallback (non-identity group_mask)
    def lrelu(x):
        return np.where(x > 0, x, ALPHA * x)

    def mlp(x, W1, b1, W2, b2, W3, b3):
        h = lrelu(x @ W1 + b1)
        h = h + lrelu(h @ W2 + b2)
        return h @ W3 + b3

    n = W.shape[0]
    W_adj = W * (1.0 - np.eye(n, dtype=W.dtype))
    Xm = X[:, None, :] * group_mask
    E = np.broadcast_to(embeddings, (X.shape[0], n, embeddings.shape[1]))
    Xe = mlp(np.concatenate([Xm, E], 2), g_W1, g_b1, g_W2, g_b2, g_W3, g_b3)
    Xa = np.einsum("ji,bjd->bid", W_adj, Xe)
    Xr = mlp(np.concatenate([Xa, E], 2), f_W1, f_b1, f_W2, f_b2, f_W3, f_b3)
    return (Xr * group_mask).sum(axis=1).astype(np.float32)


def kernel(X, W, embeddings, g_W1, g_b1, g_W2, g_b2, g_W3, g_b3,
           f_W1, f_b1, f_W2, f_b2, f_W3, f_b3, group_mask, _run_kw=None):
    if not np.allclose(group_mask, np.eye(N, D, dtype=np.float32)):
        return _kernel_numpy(X, W, embeddings, g_W1, g_b1, g_W2, g_b2, g_W3,
                             g_b3, f_W1, f_b1, f_W2, f_b2, f_W3, f_b3,
                             group_mask)

    from concourse import bass_utils

    zero_b2 = not (np.any(g_b2) or np.any(f_b2))
    consts = _host_consts(W, embeddings, g_W1, g_b1, g_W2, g_b2, g_W3, g_b3,
                          f_W1, f_b1, f_W2, f_b2, f_W3, f_b3)
    XT = _bf16(np.asarray(X, np.float32).T)  # [N, B] bf16
    in_maps = []
    for k in range(NCORES):
        m = dict(consts)
        m["XT"] = np.ascontiguousarray(XT[:, k * BC : (k + 1) * BC])
        in_maps.append(m)

    nc = _get_program(zero_b2)
    res = bass_utils.run_bass_kernel_spmd(
        nc, in_maps, core_ids=list(range(NCORES)), **(_run_kw or {})
    )
    out = np.empty((B, D), np.float32)
    for k in range(NCORES):
        out[k * BC : (k + 1) * BC, :] = res.results[k]["OUT"].T
    out += f_b3.reshape(1, D).astype(np.float32)
    if _run_kw:
        kernel.last_results = res
    return out


# revision 21
# speedup vs baseline: 1.1985x; 1.1985x over previous
"""Trainium2 Bass kernel for ContractiveInvertibleGNN feed-forward.

Math (reference, with group_mask == I_32):
  out[b,i] = f_i( sum_j W_adj[j,i] * g_j(X[b,j]) )
where g_j: R -> R^32 and f_i: R^32 -> R are slices of two shared MLPs
(64->128->128->32 with a residual middle block, LeakyReLU 0.01):
  g: H1 = lrelu(X[b,j]*U_j + C1_j); H2 = H1 + lrelu(H1@W2g + b2g)
     X_emb = H2 @ W3g + b3g
  f: Hf1 = lrelu(X_aggr@Wf1x + C2_i); Hf2 = Hf1 + lrelu(Hf1@Wf2 + bf2)
     out_i = Hf2 . V_i (+ bf3_i)
with per-node constants U_j = g_W1[j,:], C1_j = emb_j@g_W1[32:]+g_b1,
C2_i = emb_i@f_W1[32:]+f_b1 (+ (sum_j W_adj[j,i])*g_b3@f_W1[:32]),
V_i = f_W3[:,i].

Sharding: pure data-parallel over batch across 8 cores (2048 rows each).

Engine balance (the point of this implementation): activations are bf16 in
SBUF (psum stays f32). LeakyReLU work is split by column ranges across the
Activation engine (fused scale/bias lrelu), DVE (mult by alpha) and Pool
(max) so no single engine serializes the kernel. The f-phase residual
H2f = Hf1 + lrelu(...) is materialized with one DVE add so the final V-dot
needs 4 accumulating matmuls instead of 8. Matmuls run on bf16 operands
(1 cycle/row, same as f32r) which also halves DMA and removes the
f32->f32r staging copies of the earlier version.

On-chip layout (per core): node-major columns. g-phase runs per node j over
[128, 2048] tiles; X_emb assembled as Xe[(c,d), (j,t)] with c = batch
quarter stacked on partition groups; DVE transpose -> Xt[(c,j),(t,d)];
block-diag(W_adj) matmul aggregates over j; DVE transpose back ->
Xa[(c,d),(i,t)]; f-phase per node i with padded stationaries selecting
partition group c; final dot with V_i via a [128,32] stationary that also
routes batch quarter c to psum row c.
"""

import os
import sys

import numpy as np

for _p in ("/opt/trn_rl_repo", "/root/.axon_site/_ro/trn_rl_repo"):
    if os.path.isdir(_p) and _p not in sys.path:
        sys.path.insert(0, _p)

N = 32          # nodes
D = 32          # processed dim (== N, group_mask = I)
A = 128         # hidden width
B = 16384       # batch
NCORES = 8
BC = B // NCORES        # 2048 rows per core
CH = 512                # matmul free-dim chunk
NCH = BC // CH          # 4 chunks (partition-group stacking factor)
ALPHA = 0.01

# Column splits: how many of each per-node activation's 2048 columns run on
# the Activation engine. The rest: h1's tail runs as DVE z=x*u+c1, Pool
# n=(alpha-1)*min(z,0) (SBUF-sourced; GPSIMD cannot touch PSUM or do float
# max), then a DVE/Pool split of the final add. t2/tf tails run as DVE
# mult+max pairs or DVE-copy + Pool min/add chains. The f-phase residual
# Hf2 = hf1 + tf is materialized on DVE so V-dot needs only 4 accumulating
# matmuls per node, packed 8 nodes per PSUM tile.
H1A = 832               # h1 lrelu Act cols; rest DVE-z + Pool min + add
H1PD = 816              # of the h1 tail adds, cols done by DVE (rest Pool)
T2A = 1792              # t2 lrelu Act cols; rest DVE mult+max pair
TFA = 1152              # tf lrelu Act cols; rest DVE-copy + Pool min/add
NVP = 8                 # output nodes sharing one V-dot psum tile


def _build_program(zero_b2=True):
    from contextlib import ExitStack

    from concourse import bacc, mybir, tile

    f32 = mybir.dt.float32
    bf16 = mybir.dt.bfloat16
    LRELU = mybir.ActivationFunctionType.Lrelu
    ALU_MULT = mybir.AluOpType.mult
    ALU_ADD = mybir.AluOpType.add
    ALU_MAX = mybir.AluOpType.max
    ALU_MIN = mybir.AluOpType.min

    nc = bacc.Bacc("TRN2", target_bir_lowering=False, debug=False)

    def din(name, shape, dt):
        return nc.dram_tensor(
            name, list(shape), dt, kind="ExternalInput"
        ).ap()

    xt_d = din("XT", (N, BC), bf16)
    gw2_d = din("GW2", (A, A), bf16)
    fw2_d = din("FW2", (A, A), bf16)
    gw3p_d = din("GW3P", (A, NCH * A), bf16)   # col-block c: g_W3 at cols 32c..
    fw1p_d = din("FW1P", (A, NCH * A), bf16)   # row-block c: f_W1[:32] rows 32c..
    bd_d = din("BD", (A, A), bf16)             # kron(I4, W_adj)
    u_d = din("U", (A, N), f32)
    c1_d = din("C1", (A, N), f32)
    c2_d = din("C2", (A, N), f32)
    gb2_d = din("GB2", (A, 1), f32)
    fb2_d = din("FB2", (A, 1), f32)
    # V-dot stationary: slice (i,c) = cols [(i*NCH+c)*D, +D) with V_i at
    # column 4*(i%NVP)+c so NVP nodes' dots accumulate into one psum tile.
    vp_d = din("VP2", (A, N * NCH * D), bf16)
    out_d = nc.dram_tensor("OUT", [N, BC], f32, kind="ExternalOutput").ap()

    HCH = 2 * CH        # 1024: half of a node's batch columns
    T2AH = T2A // 2     # Act cols of t2 per half
    TFAH = TFA // 2     # Act cols of tf per half
    H1T = BC - H1A      # h1 tail cols (DVE/Pool path)

    with tile.TileContext(nc) as tc, ExitStack() as ctx:
        const = ctx.enter_context(tc.tile_pool(name="const", bufs=1))
        xep = ctx.enter_context(tc.tile_pool(name="xep", bufs=1))
        workp = ctx.enter_context(tc.tile_pool(name="work", bufs=3))
        scrp = ctx.enter_context(tc.tile_pool(name="scr", bufs=3))
        outp = ctx.enter_context(tc.tile_pool(name="outs", bufs=2))
        ppA = ctx.enter_context(tc.tile_pool(name="ppA", bufs=2, space="PSUM"))
        ppB = ctx.enter_context(tc.tile_pool(name="ppB", bufs=2, space="PSUM"))
        ppR = ctx.enter_context(tc.tile_pool(name="ppR", bufs=2, space="PSUM"))

        def load_const(ap_dram, shape):
            t = const.tile(list(shape), ap_dram.dtype,
                           tag=f"c_{ap_dram.tensor.name}")
            nc.sync.dma_start(t[:, :], ap_dram)
            return t

        gw2_s = load_const(gw2_d, (A, A))
        fw2_s = load_const(fw2_d, (A, A))
        gw3p_s = load_const(gw3p_d, (A, NCH * A))
        fw1p_s = load_const(fw1p_d, (A, NCH * A))
        bd_s = load_const(bd_d, (A, A))
        u_s = load_const(u_d, (A, N))
        c1_s = load_const(c1_d, (A, N))
        c2_s = load_const(c2_d, (A, N))
        gb2_s = load_const(gb2_d, (A, 1))
        fb2_s = load_const(fb2_d, (A, 1))
        vp2_s = load_const(vp_d, (A, N * NCH * D))

        # Xe[(c,d), (j,t)] = X_emb[d, j, c*CH+t]
        xe = xep.tile([A, N * CH], bf16, tag="xe")

        # ---------------- g phase ----------------
        # h1 production runs one node ahead of the matmul pipeline so the
        # DVE->Pool h1-tail chain never sits on the critical path, and each
        # engine's in-order stream always has ready work queued.
        xbc_tiles = {}
        h1_tiles = {}

        def emit_xbc(j):
            xbc = workp.tile([A, BC], bf16, tag="xbc")
            nc.sync.dma_start(
                xbc[:, :], xt_d[j : j + 1, :].partition_broadcast(A)
            )
            xbc_tiles[j] = xbc

        def emit_h1(j):
            xbc = xbc_tiles.pop(j)
            h1 = workp.tile([A, BC], bf16, tag="h1")
            nc.scalar.activation(
                h1[:, :H1A], xbc[:, :H1A], LRELU,
                bias=c1_s[:, j : j + 1], scale=u_s[:, j : j + 1], alpha=ALPHA,
            )
            # DVE: z = x*u + c1 (bf16). Pool (no PSUM access, no float max):
            # n = (alpha-1)*min(z,0). lrelu = z + n, add split DVE/Pool.
            zt = scrp.tile([A, H1T], bf16, tag="zt")
            mt = scrp.tile([A, H1T], bf16, tag="mt")
            nc.vector.tensor_scalar(zt[:, :], xbc[:, H1A:],
                                    u_s[:, j : j + 1], c1_s[:, j : j + 1],
                                    ALU_MULT, ALU_ADD)
            nc.gpsimd.tensor_scalar(mt[:, :], zt[:, :], 0.0, ALPHA - 1.0,
                                    ALU_MIN, ALU_MULT)
            nc.vector.tensor_tensor(h1[:, H1A : H1A + H1PD], zt[:, :H1PD],
                                    mt[:, :H1PD], ALU_ADD)
            nc.gpsimd.tensor_tensor(h1[:, H1A + H1PD :], zt[:, H1PD:],
                                    mt[:, H1PD:], ALU_ADD)
            h1_tiles[j] = h1

        emit_xbc(0)
        emit_xbc(1)
        emit_h1(0)
        for j in range(N):
            # stay ahead: DMA two nodes out, h1 one node out
            if j + 2 < N:
                emit_xbc(j + 2)
            if j + 1 < N:
                emit_h1(j + 1)
            h1 = h1_tiles.pop(j)
            t2 = workp.tile([A, BC], bf16, tag="t2")
            for h in range(2):  # halves of 1024 cols
                pa = ppA.tile([A, HCH], f32, tag="pA")
                for q in range(2):
                    sl = slice(h * HCH + q * CH, h * HCH + (q + 1) * CH)
                    nc.tensor.matmul(
                        pa[:, q * CH : (q + 1) * CH], gw2_s[:, :],
                        h1[:, sl], start=True, stop=True,
                    )
                off = h * HCH
                if T2AH:
                    nc.scalar.activation(
                        t2[:, off : off + T2AH], pa[:, :T2AH], LRELU,
                        bias=gb2_s[:, 0:1], alpha=ALPHA,
                    )
                pcols = HCH - T2AH
                if pcols:
                    # PSUM-sourced tail: DVE-only pair (GPSIMD can't read
                    # PSUM).
                    m2 = scrp.tile([A, pcols], bf16, tag="m2")
                    if zero_b2:
                        nc.vector.tensor_scalar(
                            m2[:, :], pa[:, T2AH:], ALPHA, None, ALU_MULT)
                        nc.vector.tensor_tensor(
                            t2[:, off + T2AH : off + HCH], pa[:, T2AH:],
                            m2[:, :], ALU_MAX)
                    else:
                        z2 = scrp.tile([A, pcols], bf16, tag="z2")
                        nc.vector.tensor_scalar(
                            z2[:, :], pa[:, T2AH:], gb2_s[:, 0:1], None,
                            ALU_ADD)
                        nc.gpsimd.tensor_scalar(
                            m2[:, :], z2[:, :], 0.0, ALPHA - 1.0,
                            ALU_MIN, ALU_MULT)
                        nc.gpsimd.tensor_tensor(
                            t2[:, off + T2AH : off + HCH], z2[:, :],
                            m2[:, :], ALU_ADD)
            # X_emb = g_W3^T @ (H1 + lrelu(.)) via 8 accumulating matmuls,
            # chunk c routed to psum rows 32c by the padded stationary.
            # h1 passes first: they are ready before t2 lands.
            pm3 = ppB.tile([A, CH], f32, tag="pB")
            for c in range(NCH):
                nc.tensor.matmul(pm3[:, :], gw3p_s[:, c * A : (c + 1) * A],
                                 h1[:, c * CH : (c + 1) * CH],
                                 start=(c == 0), stop=False)
            for c in range(NCH):
                nc.tensor.matmul(pm3[:, :], gw3p_s[:, c * A : (c + 1) * A],
                                 t2[:, c * CH : (c + 1) * CH],
                                 start=False, stop=(c == NCH - 1))
            nc.vector.tensor_copy(xe[:, j * CH : (j + 1) * CH], pm3[:, :])

        # ---------------- aggregation ----------------
        # T1: Xe[(c,d),(j,t)] -> Xt[(c,j),(t,d)]
        xt3 = xe.rearrange("p (j t) -> p j t", j=N).transpose([0, 2, 1])
        xtile = xep.tile([A, CH * D], bf16, tag="xt")
        xto = xtile.rearrange("p (t d) -> p t d", d=D)
        TS = 8  # split into 8 ops for overlap
        tstep = CH // TS
        for s in range(TS):
            nc.vector.transpose(
                xto[:, s * tstep : (s + 1) * tstep, :],
                xt3[:, s * tstep : (s + 1) * tstep, :],
            )
        # agg windows + T2-back: psum[(c,i),(t16,d)] -> Xa[(c,d),(i,t)].
        # StreamTranspose can't convert dtypes, so the idle Act engine
        # copies psum f32 -> bf16 first (Copy shares the Lrelu table).
        COPY = mybir.ActivationFunctionType.Copy
        xa = xep.tile([A, N * CH], bf16, tag="xa")
        xa3 = xa.rearrange("p (i t) -> p i t", i=N).transpose([0, 2, 1])
        WT = CH // D  # 16 t per window
        for w in range(CH // WT):  # 32 windows
            pg = ppB.tile([A, CH], f32, tag="pB")
            nc.tensor.matmul(
                pg[:, :], bd_s[:, :],
                xtile[:, w * CH : (w + 1) * CH], start=True, stop=True,
            )
            pgb = scrp.tile([A, CH], bf16, tag="pgb")
            nc.scalar.activation(pgb[:, :], pg[:, :], COPY)
            nc.vector.transpose(
                xa3[:, w * WT : (w + 1) * WT, :],
                pgb.rearrange("p (t d) -> p t d", d=D)[:, :, :],
            )

        # ---------------- f phase: one node i per iteration ----------------
        # V-dot psum packing: NVP nodes share one [32, CH] psum tile; the
        # VP2 stationary routes node i chunk c to psum row 4*(i%NVP)+c.
        pr = None
        for i in range(N):
            rhs = xa[:, i * CH : (i + 1) * CH]
            hf1 = workp.tile([A, BC], bf16, tag="hf1")
            for h in range(2):
                pa = ppA.tile([A, HCH], f32, tag="pA")
                for q in range(2):
                    c = h * 2 + q
                    nc.tensor.matmul(
                        pa[:, q * CH : (q + 1) * CH],
                        fw1p_s[:, c * A : (c + 1) * A], rhs,
                        start=True, stop=True,
                    )
                nc.scalar.activation(
                    hf1[:, h * HCH : (h + 1) * HCH], pa[:, :], LRELU,
                    bias=c2_s[:, i : i + 1], alpha=ALPHA,
                )
            tf = workp.tile([A, BC], bf16, tag="tf")
            for h in range(2):
                pa = ppA.tile([A, HCH], f32, tag="pA")
                for q in range(2):
                    c = h * 2 + q
                    nc.tensor.matmul(
                        pa[:, q * CH : (q + 1) * CH], fw2_s[:, :],
                        hf1[:, c * CH : (c + 1) * CH], start=True, stop=True,
                    )
                off = h * HCH
                if TFAH:
                    nc.scalar.activation(
                        tf[:, off : off + TFAH], pa[:, :TFAH], LRELU,
                        bias=fb2_s[:, 0:1], alpha=ALPHA,
                    )
                pcols = HCH - TFAH
                if pcols:
                    # PSUM tail: DVE copies psum->bf16 (adding fb2 when
                    # nonzero), Pool finishes lrelu as z+(alpha-1)*min(z,0).
                    zf = scrp.tile([A, pcols], bf16, tag="zf")
                    mf = scrp.tile([A, pcols], bf16, tag="mf")
                    if zero_b2:
                        nc.vector.tensor_copy(zf[:, :], pa[:, TFAH:])
                    else:
                        nc.vector.tensor_scalar(
                            zf[:, :], pa[:, TFAH:], fb2_s[:, 0:1], None,
                            ALU_ADD)
                    nc.gpsimd.tensor_scalar(
                        mf[:, :], zf[:, :], 0.0, ALPHA - 1.0,
                        ALU_MIN, ALU_MULT)
                    nc.gpsimd.tensor_tensor(
                        tf[:, off + TFAH : off + HCH], zf[:, :],
                        mf[:, :], ALU_ADD)
            # Residual add on DVE (bf16 SBUF, 2x mode).
            hf2 = workp.tile([A, BC], bf16, tag="hf2")
            nc.vector.tensor_tensor(hf2[:, :], hf1[:, :], tf[:, :], ALU_ADD)
            if i % NVP == 0:
                pr = ppR.tile([D, CH], f32, tag="pR")
            for c in range(NCH):
                base = (i * NCH + c) * D
                nc.tensor.matmul(pr[:, :], vp2_s[:, base : base + D],
                                 hf2[:, c * CH : (c + 1) * CH],
                                 start=(i % NVP == 0 and c == 0),
                                 stop=(i % NVP == NVP - 1 and c == NCH - 1))
            if i % NVP == NVP - 1:
                g0 = i - (NVP - 1)
                osb = outp.tile([NVP * NCH, CH], f32, tag="o")
                nc.vector.tensor_copy(osb[:, :], pr[: NVP * NCH, :])
                nc.sync.dma_start(
                    out_d[g0 : g0 + NVP, :].rearrange(
                        "o (c t) -> (o c) t", c=NCH),
                    osb[:, :],
                )

    nc.compile()
    return nc


_NC_CACHE = {}


def _get_program(zero_b2=True):
    if zero_b2 not in _NC_CACHE:
        _NC_CACHE[zero_b2] = _build_program(zero_b2)
    return _NC_CACHE[zero_b2]


def _bf16(x):
    import ml_dtypes
    return np.asarray(x, np.float32).astype(ml_dtypes.bfloat16)


def _host_consts(W, embeddings, g_W1, g_b1, g_W2, g_b2, g_W3, g_b3,
                 f_W1, f_b1, f_W2, f_b2, f_W3, f_b3):
    f = np.float32
    W_adj = (W * (1.0 - np.eye(N, dtype=f))).astype(f)
    U = np.ascontiguousarray(g_W1[:D].T, dtype=f)                    # [A, N]
    C1 = np.ascontiguousarray((embeddings @ g_W1[D:] + g_b1).T, f)   # [A, N]
    s = W_adj.sum(axis=0)                                            # [N]
    C2 = (embeddings @ f_W1[D:] + f_b1 + np.outer(s, g_b3 @ f_W1[:D]))
    C2 = np.ascontiguousarray(C2.T, dtype=f)                         # [A, N]
    GW3P = np.zeros((A, NCH * A), f)
    FW1P = np.zeros((A, NCH * A), f)
    for c in range(NCH):
        GW3P[:, c * A + c * D : c * A + (c + 1) * D] = g_W3
        FW1P[c * D : (c + 1) * D, c * A : (c + 1) * A] = f_W1[:D]
    BD = np.kron(np.eye(NCH, dtype=f), W_adj).astype(f)
    VP2 = np.zeros((A, N * NCH * D), f)
    for i in range(N):
        for c in range(NCH):
            VP2[:, (i * NCH + c) * D + NCH * (i % NVP) + c] = f_W3[:, i]
    return {
        "GW2": _bf16(g_W2),
        "FW2": _bf16(f_W2),
        "GW3P": _bf16(GW3P), "FW1P": _bf16(FW1P), "BD": _bf16(BD),
        "U": U, "C1": C1, "C2": C2,
        "GB2": np.ascontiguousarray(g_b2.reshape(A, 1), f),
        "FB2": np.ascontiguousarray(f_b2.reshape(A, 1), f),
        "VP2": _bf16(VP2),
    }


def _kernel_numpy(X, W, embeddings, g_W1, g_b1, g_W2, g_b2, g_W3, g_b3,
                  f_W1, f_b1, f_W2, f_b2, f_W3, f_b3, group_mask):
    # general fallback (non-identity group_mask)
    def lrelu(x):
        return np.where(x > 0, x, ALPHA * x)

    def mlp(x, W1, b1, W2, b2, W3, b3):
        h = lrelu(x @ W1 + b1)
        h = h + lrelu(h @ W2 + b2)
        return h @ W3 + b3

    n = W.shape[0]
    W_adj = W * (1.0 - np.eye(n, dtype=W.dtype))
    Xm = X[:, None, :] * group_mask
    E = np.broadcast_to(embeddings, (X.shape[0], n, embeddings.shape[1]))
    Xe = mlp(np.concatenate([Xm, E], 2), g_W1, g_b1, g_W2, g_b2, g_W3, g_b3)
    Xa = np.einsum("ji,bjd->bid", W_adj, Xe)
    Xr = mlp(np.concatenate([Xa, E], 2), f_W1, f_b1, f_W2, f_b2, f_W3, f_b3)
    return (Xr * group_mask).sum(axis=1).astype(np.float32)


def kernel(X, W, embeddings, g_W1, g_b1, g_W2, g_b2, g_W3, g_b3,
           f_W1, f_b1, f_W2, f_b2, f_W3, f_b3, group_mask, _run_kw=None):
    if not np.allclose(group_mask, np.eye(N, D, dtype=np.float32)):
        return _kernel_numpy(X, W, embeddings, g_W1, g_b1, g_W2, g_b2, g_W3,
                             g_b3, f_W1, f_b1, f_W2, f_b2, f_W3, f_b3,
                             group_mask)

    from concourse import bass_utils

    zero_b2 = not (np.any(g_b2) or np.any(f_b2))
    consts = _host_consts(W, embeddings, g_W1, g_b1, g_W2, g_b2, g_W3, g_b3,
                          f_W1, f_b1, f_W2, f_b2, f_W3, f_b3)
    XT = _bf16(np.asarray(X, np.float32).T)  # [N, B] bf16
    in_maps = []
    for k in range(NCORES):
        m = dict(consts)
        m["XT"] = np.ascontiguousarray(XT[:, k * BC : (k + 1) * BC])
        in_maps.append(m)

    nc = _get_program(zero_b2)
    res = bass_utils.run_bass_kernel_spmd(
        nc, in_maps, core_ids=list(range(NCORES)), **(_run_kw or {})
    )
    out = np.empty((B, D), np.float32)
    for k in range(NCORES):
        out[k * BC : (k + 1) * BC, :] = res.results[k]["OUT"].T
    out += f_b3.reshape(1, D).astype(np.float32)
    if _run_kw:
        kernel.last_results = res
    return out


# revision 25
# speedup vs baseline: 1.2606x; 1.0518x over previous
"""Trainium2 Bass kernel for ContractiveInvertibleGNN feed-forward.

Math (reference, with group_mask == I_32):
  out[b,i] = f_i( sum_j W_adj[j,i] * g_j(X[b,j]) )
where g_j: R -> R^32 and f_i: R^32 -> R are slices of two shared MLPs
(64->128->128->32 with a residual middle block, LeakyReLU 0.01):
  g: H1 = lrelu(X[b,j]*U_j + C1_j); H2 = H1 + lrelu(H1@W2g + b2g)
     X_emb = H2 @ W3g + b3g
  f: Hf1 = lrelu(X_aggr@Wf1x + C2_i); Hf2 = Hf1 + lrelu(Hf1@Wf2 + bf2)
     out_i = Hf2 . V_i (+ bf3_i)
with per-node constants U_j = g_W1[j,:], C1_j = emb_j@g_W1[32:]+g_b1,
C2_i = emb_i@f_W1[32:]+f_b1 (+ (sum_j W_adj[j,i])*g_b3@f_W1[:32]),
V_i = f_W3[:,i].

Sharding: pure data-parallel over batch across 8 cores (2048 rows each).

Engine balance (the point of this implementation): activations are bf16 in
SBUF (psum stays f32). LeakyReLU work is split by column ranges across the
Activation engine (fused scale/bias lrelu), DVE (mult by alpha) and Pool
(max) so no single engine serializes the kernel. The f-phase residual
H2f = Hf1 + lrelu(...) is materialized with one DVE add so the final V-dot
needs 4 accumulating matmuls instead of 8. Matmuls run on bf16 operands
(1 cycle/row, same as f32r) which also halves DMA and removes the
f32->f32r staging copies of the earlier version.

On-chip layout (per core): node-major columns. g-phase runs per node j over
[128, 2048] tiles; X_emb assembled as Xe[(c,d), (j,t)] with c = batch
quarter stacked on partition groups; DVE transpose -> Xt[(c,j),(t,d)];
block-diag(W_adj) matmul aggregates over j; DVE transpose back ->
Xa[(c,d),(i,t)]; f-phase per node i with padded stationaries selecting
partition group c; final dot with V_i via a [128,32] stationary that also
routes batch quarter c to psum row c.
"""

import os
import sys

import numpy as np

for _p in ("/opt/trn_rl_repo", "/root/.axon_site/_ro/trn_rl_repo"):
    if os.path.isdir(_p) and _p not in sys.path:
        sys.path.insert(0, _p)

N = 32          # nodes
D = 32          # processed dim (== N, group_mask = I)
A = 128         # hidden width
B = 16384       # batch
NCORES = 8
BC = B // NCORES        # 2048 rows per core
CH = 512                # matmul free-dim chunk
NCH = BC // CH          # 4 chunks (partition-group stacking factor)
ALPHA = 0.01

# Column splits: how many of each per-node activation's 2048 columns run on
# the Activation engine. The rest: h1's tail runs as DVE z=x*u+c1, Pool
# n=(alpha-1)*min(z,0) (SBUF-sourced; GPSIMD cannot touch PSUM or do float
# max), then a DVE/Pool split of the final add. t2/tf tails run as DVE
# mult+max pairs or DVE-copy + Pool min/add chains. The f-phase residual
# Hf2 = hf1 + tf is materialized on DVE so V-dot needs only 4 accumulating
# matmuls per node, packed 8 nodes per PSUM tile.
H1A = 832               # h1 lrelu Act cols; rest DVE-z + Pool min + add
H1PD = 816              # of the h1 tail adds, cols done by DVE (rest Pool)
T2A = 1792              # t2 lrelu Act cols; rest DVE mult+max pair
TFA = 1152              # tf lrelu Act cols; rest DVE-copy + Pool min/add
NVP = 8                 # output nodes sharing one V-dot psum tile


def _build_program(zero_b2=True):
    from contextlib import ExitStack

    from concourse import bacc, mybir, tile

    f32 = mybir.dt.float32
    bf16 = mybir.dt.bfloat16
    LRELU = mybir.ActivationFunctionType.Lrelu
    ALU_MULT = mybir.AluOpType.mult
    ALU_ADD = mybir.AluOpType.add
    ALU_MAX = mybir.AluOpType.max
    ALU_MIN = mybir.AluOpType.min

    nc = bacc.Bacc("TRN2", target_bir_lowering=False, debug=False)

    def din(name, shape, dt):
        return nc.dram_tensor(
            name, list(shape), dt, kind="ExternalInput"
        ).ap()

    xt_d = din("XT", (N, BC), bf16)
    gw2_d = din("GW2", (A, A), bf16)
    fw2_d = din("FW2", (A, A), bf16)
    gw3p_d = din("GW3P", (A, NCH * A), bf16)   # col-block c: g_W3 at cols 32c..
    fw1p_d = din("FW1P", (A, NCH * A), bf16)   # row-block c: f_W1[:32] rows 32c..
    bd_d = din("BD", (A, A), bf16)             # kron(I4, W_adj)
    u_d = din("U", (A, N), f32)
    c1_d = din("C1", (A, N), f32)
    c2_d = din("C2", (A, N), f32)
    gb2_d = din("GB2", (A, 1), f32)
    fb2_d = din("FB2", (A, 1), f32)
    # V-dot stationary: slice (i,c) = cols [(i*NCH+c)*D, +D) with V_i at
    # column 4*(i%NVP)+c so NVP nodes' dots accumulate into one psum tile.
    vp_d = din("VP2", (A, N * NCH * D), bf16)
    out_d = nc.dram_tensor("OUT", [N, BC], f32, kind="ExternalOutput").ap()

    HCH = 2 * CH        # 1024: half of a node's batch columns
    T2AH = T2A // 2     # Act cols of t2 per half
    TFAH = TFA // 2     # Act cols of tf per half
    H1T = BC - H1A      # h1 tail cols (DVE/Pool path)

    with tile.TileContext(nc) as tc, ExitStack() as ctx:
        const = ctx.enter_context(tc.tile_pool(name="const", bufs=1))
        xep = ctx.enter_context(tc.tile_pool(name="xep", bufs=1))
        workp = ctx.enter_context(tc.tile_pool(name="work", bufs=2))
        scrp = ctx.enter_context(tc.tile_pool(name="scr", bufs=3))
        outp = ctx.enter_context(tc.tile_pool(name="outs", bufs=2))
        # PSUM: ppA 2x2 banks + ppB 3x1 + ppR 1x1 = 8 banks (the full file).
        # ppR can be single-buffered because V-dot packs NVP nodes per tile.
        ppA = ctx.enter_context(tc.tile_pool(name="ppA", bufs=2, space="PSUM"))
        ppB = ctx.enter_context(tc.tile_pool(name="ppB", bufs=3, space="PSUM"))
        ppR = ctx.enter_context(tc.tile_pool(name="ppR", bufs=1, space="PSUM"))

        def load_const(ap_dram, shape):
            t = const.tile(list(shape), ap_dram.dtype,
                           tag=f"c_{ap_dram.tensor.name}")
            nc.sync.dma_start(t[:, :], ap_dram)
            return t

        gw2_s = load_const(gw2_d, (A, A))
        fw2_s = load_const(fw2_d, (A, A))
        gw3p_s = load_const(gw3p_d, (A, NCH * A))
        fw1p_s = load_const(fw1p_d, (A, NCH * A))
        bd_s = load_const(bd_d, (A, A))
        u_s = load_const(u_d, (A, N))
        c1_s = load_const(c1_d, (A, N))
        c2_s = load_const(c2_d, (A, N))
        gb2_s = load_const(gb2_d, (A, 1))
        fb2_s = load_const(fb2_d, (A, 1))
        vp2_s = load_const(vp_d, (A, N * NCH * D))

        # Xe[(c,d), (j,t)] = X_emb[d, j, c*CH+t]
        xe = xep.tile([A, N * CH], bf16, tag="xe")

        # ---------------- g phase ----------------
        # h1 production runs one node ahead of the matmul pipeline so the
        # DVE->Pool h1-tail chain never sits on the critical path, and each
        # engine's in-order stream always has ready work queued.
        xbc_tiles = {}
        h1_tiles = {}

        def emit_xbc(j):
            xbc = workp.tile([A, BC], bf16, tag="xbc", bufs=3)
            nc.sync.dma_start(
                xbc[:, :], xt_d[j : j + 1, :].partition_broadcast(A)
            )
            xbc_tiles[j] = xbc

        def emit_h1(j):
            xbc = xbc_tiles.pop(j)
            h1 = workp.tile([A, BC], bf16, tag="h1", bufs=3)
            nc.scalar.activation(
                h1[:, :H1A], xbc[:, :H1A], LRELU,
                bias=c1_s[:, j : j + 1], scale=u_s[:, j : j + 1], alpha=ALPHA,
            )
            # DVE: z = x*u + c1 (bf16). Pool (no PSUM access, no float max):
            # n = (alpha-1)*min(z,0). lrelu = z + n, add split DVE/Pool.
            zt = scrp.tile([A, H1T], bf16, tag="zt", bufs=2)
            mt = scrp.tile([A, H1T], bf16, tag="mt", bufs=2)
            nc.vector.tensor_scalar(zt[:, :], xbc[:, H1A:],
                                    u_s[:, j : j + 1], c1_s[:, j : j + 1],
                                    ALU_MULT, ALU_ADD)
            nc.gpsimd.tensor_scalar(mt[:, :], zt[:, :], 0.0, ALPHA - 1.0,
                                    ALU_MIN, ALU_MULT)
            nc.vector.tensor_tensor(h1[:, H1A : H1A + H1PD], zt[:, :H1PD],
                                    mt[:, :H1PD], ALU_ADD)
            nc.gpsimd.tensor_tensor(h1[:, H1A + H1PD :], zt[:, H1PD:],
                                    mt[:, H1PD:], ALU_ADD)
            h1_tiles[j] = h1

        emit_xbc(0)
        emit_xbc(1)
        emit_h1(0)
        for j in range(N):
            # stay ahead: DMA two nodes out, h1 one node out
            if j + 2 < N:
                emit_xbc(j + 2)
            if j + 1 < N:
                emit_h1(j + 1)
            h1 = h1_tiles.pop(j)
            t2 = workp.tile([A, BC], bf16, tag="t2")
            for h in range(2):  # halves of 1024 cols
                pa = ppA.tile([A, HCH], f32, tag="pA")
                for q in range(2):
                    sl = slice(h * HCH + q * CH, h * HCH + (q + 1) * CH)
                    nc.tensor.matmul(
                        pa[:, q * CH : (q + 1) * CH], gw2_s[:, :],
                        h1[:, sl], start=True, stop=True,
                    )
                off = h * HCH
                if T2AH:
                    nc.scalar.activation(
                        t2[:, off : off + T2AH], pa[:, :T2AH], LRELU,
                        bias=gb2_s[:, 0:1], alpha=ALPHA,
                    )
                pcols = HCH - T2AH
                if pcols:
                    # PSUM-sourced tail: DVE-only pair (GPSIMD can't read
                    # PSUM).
                    m2 = scrp.tile([A, pcols], bf16, tag="m2")
                    if zero_b2:
                        nc.vector.tensor_scalar(
                            m2[:, :], pa[:, T2AH:], ALPHA, None, ALU_MULT)
                        nc.vector.tensor_tensor(
                            t2[:, off + T2AH : off + HCH], pa[:, T2AH:],
                            m2[:, :], ALU_MAX)
                    else:
                        z2 = scrp.tile([A, pcols], bf16, tag="z2")
                        nc.vector.tensor_scalar(
                            z2[:, :], pa[:, T2AH:], gb2_s[:, 0:1], None,
                            ALU_ADD)
                        nc.gpsimd.tensor_scalar(
                            m2[:, :], z2[:, :], 0.0, ALPHA - 1.0,
                            ALU_MIN, ALU_MULT)
                        nc.gpsimd.tensor_tensor(
                            t2[:, off + T2AH : off + HCH], z2[:, :],
                            m2[:, :], ALU_ADD)
            # X_emb = g_W3^T @ (H1 + lrelu(.)) via 8 accumulating matmuls,
            # chunk c routed to psum rows 32c by the padded stationary.
            # h1 passes first: they are ready before t2 lands.
            pm3 = ppB.tile([A, CH], f32, tag="pB")
            for c in range(NCH):
                nc.tensor.matmul(pm3[:, :], gw3p_s[:, c * A : (c + 1) * A],
                                 h1[:, c * CH : (c + 1) * CH],
                                 start=(c == 0), stop=False)
            for c in range(NCH):
                nc.tensor.matmul(pm3[:, :], gw3p_s[:, c * A : (c + 1) * A],
                                 t2[:, c * CH : (c + 1) * CH],
                                 start=False, stop=(c == NCH - 1))
            nc.vector.tensor_copy(xe[:, j * CH : (j + 1) * CH], pm3[:, :])

        # ---------------- aggregation ----------------
        # T1: Xe[(c,d),(j,t)] -> Xt[(c,j),(t,d)]
        xt3 = xe.rearrange("p (j t) -> p j t", j=N).transpose([0, 2, 1])
        xtile = xep.tile([A, CH * D], bf16, tag="xt")
        xto = xtile.rearrange("p (t d) -> p t d", d=D)
        TS = 8  # split into 8 ops for overlap
        tstep = CH // TS
        for s in range(TS):
            nc.vector.transpose(
                xto[:, s * tstep : (s + 1) * tstep, :],
                xt3[:, s * tstep : (s + 1) * tstep, :],
            )
        # agg windows + T2-back: psum[(c,i),(t16,d)] -> Xa[(c,d),(i,t)].
        # StreamTranspose can't convert dtypes, so the idle Act engine
        # copies psum f32 -> bf16 first (Copy shares the Lrelu table).
        COPY = mybir.ActivationFunctionType.Copy
        xa = xep.tile([A, N * CH], bf16, tag="xa")
        xa3 = xa.rearrange("p (i t) -> p i t", i=N).transpose([0, 2, 1])
        WT = CH // D  # 16 t per window
        for w in range(CH // WT):  # 32 windows
            pg = ppB.tile([A, CH], f32, tag="pB")
            nc.tensor.matmul(
                pg[:, :], bd_s[:, :],
                xtile[:, w * CH : (w + 1) * CH], start=True, stop=True,
            )
            pgb = scrp.tile([A, CH], bf16, tag="pgb")
            nc.scalar.activation(pgb[:, :], pg[:, :], COPY)
            nc.vector.transpose(
                xa3[:, w * WT : (w + 1) * WT, :],
                pgb.rearrange("p (t d) -> p t d", d=D)[:, :, :],
            )

        # ---------------- f phase: one node i per iteration ----------------
        # V-dot psum packing: NVP nodes share one [32, CH] psum tile; the
        # VP2 stationary routes node i chunk c to psum row 4*(i%NVP)+c.
        # hf1 production runs one node ahead (same idea as the g phase).
        hf1_tiles = {}

        def emit_hf1(i):
            rhs = xa[:, i * CH : (i + 1) * CH]
            hf1 = workp.tile([A, BC], bf16, tag="hf1", bufs=3)
            for h in range(2):
                pa = ppA.tile([A, HCH], f32, tag="pA")
                for q in range(2):
                    c = h * 2 + q
                    nc.tensor.matmul(
                        pa[:, q * CH : (q + 1) * CH],
                        fw1p_s[:, c * A : (c + 1) * A], rhs,
                        start=True, stop=True,
                    )
                nc.scalar.activation(
                    hf1[:, h * HCH : (h + 1) * HCH], pa[:, :], LRELU,
                    bias=c2_s[:, i : i + 1], alpha=ALPHA,
                )
            hf1_tiles[i] = hf1

        pr = None
        emit_hf1(0)
        for i in range(N):
            if i + 1 < N:
                emit_hf1(i + 1)
            hf1 = hf1_tiles.pop(i)
            tf = workp.tile([A, BC], bf16, tag="tf")
            for h in range(2):
                pa = ppA.tile([A, HCH], f32, tag="pA")
                for q in range(2):
                    c = h * 2 + q
                    nc.tensor.matmul(
                        pa[:, q * CH : (q + 1) * CH], fw2_s[:, :],
                        hf1[:, c * CH : (c + 1) * CH], start=True, stop=True,
                    )
                off = h * HCH
                if TFAH:
                    nc.scalar.activation(
                        tf[:, off : off + TFAH], pa[:, :TFAH], LRELU,
                        bias=fb2_s[:, 0:1], alpha=ALPHA,
                    )
                pcols = HCH - TFAH
                if pcols:
                    # PSUM tail: DVE copies psum->bf16 (adding fb2 when
                    # nonzero), Pool finishes lrelu as z+(alpha-1)*min(z,0).
                    zf = scrp.tile([A, pcols], bf16, tag="zf")
                    mf = scrp.tile([A, pcols], bf16, tag="mf")
                    if zero_b2:
                        nc.vector.tensor_copy(zf[:, :], pa[:, TFAH:])
                    else:
                        nc.vector.tensor_scalar(
                            zf[:, :], pa[:, TFAH:], fb2_s[:, 0:1], None,
                            ALU_ADD)
                    nc.gpsimd.tensor_scalar(
                        mf[:, :], zf[:, :], 0.0, ALPHA - 1.0,
                        ALU_MIN, ALU_MULT)
                    nc.gpsimd.tensor_tensor(
                        tf[:, off + TFAH : off + HCH], zf[:, :],
                        mf[:, :], ALU_ADD)
            # Residual add on DVE (bf16 SBUF, 2x mode), per chunk so each
            # V-dot matmul can start as soon as its chunk's add lands.
            hf2 = workp.tile([A, BC], bf16, tag="hf2")
            if i % NVP == 0:
                pr = ppR.tile([D, CH], f32, tag="pR")
            for c in range(NCH):
                sl = slice(c * CH, (c + 1) * CH)
                nc.vector.tensor_tensor(hf2[:, sl], hf1[:, sl], tf[:, sl],
                                        ALU_ADD)
                base = (i * NCH + c) * D
                nc.tensor.matmul(pr[:, :], vp2_s[:, base : base + D],
                                 hf2[:, sl],
                                 start=(i % NVP == 0 and c == 0),
                                 stop=(i % NVP == NVP - 1 and c == NCH - 1))
            if i % NVP == NVP - 1:
                g0 = i - (NVP - 1)
                osb = outp.tile([NVP * NCH, CH], f32, tag="o")
                nc.vector.tensor_copy(osb[:, :], pr[: NVP * NCH, :])
                nc.sync.dma_start(
                    out_d[g0 : g0 + NVP, :].rearrange(
                        "o (c t) -> (o c) t", c=NCH),
                    osb[:, :],
                )

    nc.compile()
    return nc


_NC_CACHE = {}


def _get_program(zero_b2=True):
    if zero_b2 not in _NC_CACHE:
        _NC_CACHE[zero_b2] = _build_program(zero_b2)
    return _NC_CACHE[zero_b2]


def _bf16(x):
    import ml_dtypes
    return np.asarray(x, np.float32).astype(ml_dtypes.bfloat16)


def _host_consts(W, embeddings, g_W1, g_b1, g_W2, g_b2, g_W3, g_b3,
                 f_W1, f_b1, f_W2, f_b2, f_W3, f_b3):
    f = np.float32
    W_adj = (W * (1.0 - np.eye(N, dtype=f))).astype(f)
    U = np.ascontiguousarray(g_W1[:D].T, dtype=f)                    # [A, N]
    C1 = np.ascontiguousarray((embeddings @ g_W1[D:] + g_b1).T, f)   # [A, N]
    s = W_adj.sum(axis=0)                                            # [N]
    C2 = (embeddings @ f_W1[D:] + f_b1 + np.outer(s, g_b3 @ f_W1[:D]))
    C2 = np.ascontiguousarray(C2.T, dtype=f)                         # [A, N]
    GW3P = np.zeros((A, NCH * A), f)
    FW1P = np.zeros((A, NCH * A), f)
    for c in range(NCH):
        GW3P[:, c * A + c * D : c * A + (c + 1) * D] = g_W3
        FW1P[c * D : (c + 1) * D, c * A : (c + 1) * A] = f_W1[:D]
    BD = np.kron(np.eye(NCH, dtype=f), W_adj).astype(f)
    VP2 = np.zeros((A, N * NCH * D), f)
    for i in range(N):
        for c in range(NCH):
            VP2[:, (i * NCH + c) * D + NCH * (i % NVP) + c] = f_W3[:, i]
    return {
        "GW2": _bf16(g_W2),
        "FW2": _bf16(f_W2),
        "GW3P": _bf16(GW3P), "FW1P": _bf16(FW1P), "BD": _bf16(BD),
        "U": U, "C1": C1, "C2": C2,
        "GB2": np.ascontiguousarray(g_b2.reshape(A, 1), f),
        "FB2": np.ascontiguousarray(f_b2.reshape(A, 1), f),
        "VP2": _bf16(VP2),
    }


def _kernel_numpy(X, W, embeddings, g_W1, g_b1, g_W2, g_b2, g_W3, g_b3,
                  f_W1, f_b1, f_W2, f_b2, f_W3, f_b3, group_mask):
    # general fallback (non-identity group_mask)
    def lrelu(x):
        return np.where(x > 0, x, ALPHA * x)

    def mlp(x, W1, b1, W2, b2, W3, b3):
        h = lrelu(x @ W1 + b1)
        h = h + lrelu(h @ W2 + b2)
        return h @ W3 + b3

    n = W.shape[0]
    W_adj = W * (1.0 - np.eye(n, dtype=W.dtype))
    Xm = X[:, None, :] * group_mask
    E = np.broadcast_to(embeddings, (X.shape[0], n, embeddings.shape[1]))
    Xe = mlp(np.concatenate([Xm, E], 2), g_W1, g_b1, g_W2, g_b2, g_W3, g_b3)
    Xa = np.einsum("ji,bjd->bid", W_adj, Xe)
    Xr = mlp(np.concatenate([Xa, E], 2), f_W1, f_b1, f_W2, f_b2, f_W3, f_b3)
    return (Xr * group_mask).sum(axis=1).astype(np.float32)


def kernel(X, W, embeddings, g_W1, g_b1, g_W2, g_b2, g_W3, g_b3,
           f_W1, f_b1, f_W2, f_b2, f_W3, f_b3, group_mask, _run_kw=None):
    if not np.allclose(group_mask, np.eye(N, D, dtype=np.float32)):
        return _kernel_numpy(X, W, embeddings, g_W1, g_b1, g_W2, g_b2, g_W3,
                             g_b3, f_W1, f_b1, f_W2, f_b2, f_W3, f_b3,
                             group_mask)

    from concourse import bass_utils

    zero_b2 = not (np.any(g_b2) or np.any(f_b2))
    consts = _host_consts(W, embeddings, g_W1, g_b1, g_W2, g_b2, g_W3, g_b3,
                          f_W1, f_b1, f_W2, f_b2, f_W3, f_b3)
    XT = _bf16(np.asarray(X, np.float32).T)  # [N, B] bf16
    in_maps = []
    for k in range(NCORES):
        m = dict(consts)
        m["XT"] = np.ascontiguousarray(XT[:, k * BC : (k + 1) * BC])
        in_maps.append(m)

    nc = _get_program(zero_b2)
    res = bass_utils.run_bass_kernel_spmd(
        nc, in_maps, core_ids=list(range(NCORES)), **(_run_kw or {})
    )
    out = np.empty((B, D), np.float32)
    for k in range(NCORES):
        out[k * BC : (k + 1) * BC, :] = res.results[k]["OUT"].T
    out += f_b3.reshape(1, D).astype(np.float32)
    if _run_kw:
        kernel.last_results = res
    return out
